# revision 19
# baseline (speedup 1.0000x reference)
"""MemN2N block kernel for 8 TRN2 NeuronCores.

Reference computation (per batch b):
    m      = story_a @ u^T          # (M, Q)  contraction over D
    p      = softmax(m, axis=Q)     # (M, Q)  softmax over Q (free axis)
    c      = p^T @ story_c          # (Q, D)  contraction over M
    out    = c + u @ H              # (Q, D)

Sharding: data-parallel over batch B=32 -> 4 batches per core, no
collectives.  The 4 batches per core are processed interleaved so the
ACT/DVE softmax ops cover (128, 4, 64) at once.

The dominant HW cost on this part is the ~300-450ns FIXED cost per
TensorE instruction (dispatch + weight load), so the kernel minimizes
PE instruction count:

  * story_a is pre-converted to fp16 in a DRAM scratch (fast HWDGE f32
    loads + idle-GPSIMD casts + contiguous stores), then read back with
    XBAR DMA-transpose: one DMA delivers aT = (D=128, 2048 m-rows) in
    SBUF, replacing 16 PE transposes + ACT PSUM copies per chunk.
  * mm1/mm2 run on fp16 operands (1 cyc/row + fast weight load vs fp32's
    2-pass half-speed path); PSUM accumulation stays fp32, softmax
    statistics stay fp32.  Measured end-to-end relative error ~1e-3.
  * story_c is loaded as f32 (HWDGE; the SWDGE cast-DMA path is ~10x
    slower) and cast to fp16 by the otherwise-idle GPSIMD engine.

Per m-tile step (4 batches at once):
  - 4x mm1: scores[:, b, :] = aT_chunk[:, tile].T @ uT_b   (fp32 PSUM)
  - one ACT exp over (128, 4, 64) -> fp32 e  (softmax over q needs no
    max-subtraction: |scores| <~ 70 keeps exp inside fp32 range)
  - DVE reduce_sum -> Z, reciprocal, and a broadcast-multiply that
    normalizes and converts to fp16 in one op
  - 4x mm2: acc[:, b, :] += c_tile_b.T @ p_b  (fp32 PSUM accumulation)
Tail per batch: mm3 acc += H.T @ uT (the u @ H residual, same PSUM
accumulation group), PE transpose of acc -> (q, d), DMA out f32.
"""

import numpy as np

B, M, Q, D = 32, 8192, 64, 128
N_CORES = 8
BPC = B // N_CORES          # batches per core
P = 128                     # partitions / m-tile rows
NT = M // P                 # 64 m-subtiles per batch
CHUNK = 16                  # m-subtiles per chunk (2048 rows)
ROWS = CHUNK * P            # rows per chunk
NCH = NT // CHUNK           # chunks per batch

_cache = {}


def _emit_body(nc, tc, pools, aps, variant="full", cdt_name="f16"):
    """Emit one full pass of the per-core computation.

    variant: "full" | "dma" | "mm1" (skip softmax+mm2) | "sm" (skip mm2)
    — non-full variants exist only for benchmark attribution.
    """
    from concourse import mybir

    f32 = mybir.dt.float32
    cdt = mybir.dt.float16 if cdt_name == "f16" else mybir.dt.bfloat16
    Exp = mybir.ActivationFunctionType.Exp
    consts, io, work, small, dram, ps_t, ps_s, ps_acc = pools
    a_ap, u_ap, c_ap, h_ap, o_ap, ident_f, h_sb = aps

    # u tiles and their transposes uT_b (D on partitions), loaded once
    uT_sbs = []
    for b in range(BPC):
        u_sb = small.tile([Q, D], f32, tag="u_sb")
        nc.sync.dma_start(u_sb, u_ap[b])
        uT_ps = ps_t.tile([D, Q], f32, tag="misc_ps")
        nc.tensor.transpose(uT_ps, u_sb, ident_f[:Q, :Q])
        uT_sb = small.tile([D, Q], cdt, tag=f"uT_sb{b}")
        nc.vector.tensor_copy(uT_sb, uT_ps)
        uT_sbs.append(uT_sb)

    # ---- pre-pass: story_a f32 -> fp16 DRAM scratch, per (batch, chunk).
    # Views use the same "(p n) d" split on both sides, so scratch row
    # order == DRAM row order (identity permutation), per-partition
    # contiguous reads (8KB) and writes (4KB).
    scratch = {}
    for ch in range(NCH):
        for b in range(BPC):
            src = a_ap[b][ch * ROWS:(ch + 1) * ROWS, :].rearrange(
                "(p n) d -> p n d", p=P)
            raw = io.tile([P, CHUNK, D], f32, tag="praw")
            nc.sync.dma_start(raw, src)
            a16 = io.tile([P, CHUNK, D], cdt, tag="p16")
            nc.gpsimd.tensor_copy(a16, raw)
            sc = dram.tile([ROWS, D], cdt, name=f"a16_{b}_{ch}")
            nc.sync.dma_start(
                sc.rearrange("(p n) d -> p n d", p=P), a16)
            scratch[(b, ch)] = sc

    acc_ps = ps_acc.tile([P, BPC, Q], f32, tag="acc")

    # Software pipeline: iteration i emits mm1(i) | softmax(i-1) |
    # mm2(i-2), so the in-order PE queue never waits on the ACT/DVE
    # softmax chain.
    chunk_tiles = {}      # ch -> (aTs, c16s)
    s_pss = {}            # idx -> scores PSUM
    p_sbs = {}            # idx -> normalized fp16 softmax in SBUF

    def load_chunk(ch):
        aTs, c16s = [], []
        for b in range(BPC):
            aT = io.tile([D, ROWS], cdt, tag=f"aT{b}")
            nc.sync.dma_start_transpose(aT, scratch[(b, ch)][:])
            aTs.append(aT)
            # story_c rows in tile-major order: tile j = rows
            # [ch*ROWS + j*128, +128) on partitions, matching aT's
            # column order.
            csrc = c_ap[b][ch * ROWS:(ch + 1) * ROWS, :].rearrange(
                "(n p) d -> p n d", p=P)
            cch = io.tile([P, CHUNK, D], f32, tag=f"cch{b}")
            nc.sync.dma_start(cch, csrc)
            c16 = io.tile([P, CHUNK, D], cdt, tag=f"c16_{b}")
            nc.gpsimd.tensor_copy(c16, cch)
            c16s.append(c16)
        chunk_tiles[ch] = (aTs, c16s)

    def stage_scores(idx):
        aTs, _ = chunk_tiles[idx // CHUNK]
        j = idx % CHUNK
        s_ps = ps_s.tile([P, BPC, Q], f32, tag="s_ps")
        for b in range(BPC):
            nc.tensor.matmul(s_ps[:, b, :], aTs[b][:, j * P:(j + 1) * P],
                             uT_sbs[b], start=True, stop=True)
        s_pss[idx] = s_ps

    def stage_softmax(idx):
        s_ps = s_pss.pop(idx)
        if variant == "mm1":
            return
        # exp and row-sums stay fp32 (raw exp(s) reaches ~e^70, far
        # beyond fp16 range); the normalize multiply converts to fp16.
        e_sb = work.tile([P, BPC, Q], f32, tag="e_sb")
        nc.scalar.activation(e_sb, s_ps, Exp)
        z = small.tile([P, BPC], f32, tag="z")
        nc.vector.tensor_reduce(z, e_sb, mybir.AxisListType.X,
                                mybir.AluOpType.add)
        zi = small.tile([P, BPC], f32, tag="zi")
        nc.vector.reciprocal(zi, z)
        p_sb = work.tile([P, BPC, Q], cdt, tag="p_sb")
        nc.vector.tensor_tensor(p_sb, e_sb,
                                zi[:, :, None].to_broadcast(p_sb.shape),
                                mybir.AluOpType.mult)
        p_sbs[idx] = p_sb

    def stage_weighted_sum(idx):
        _, c16s = chunk_tiles[idx // CHUNK]
        j = idx % CHUNK
        p_sb = p_sbs.pop(idx)
        for b in range(BPC):
            # PSUM accumulation-group state is per bank: only the very
            # first matmul on this bank may use start=True (interleaved
            # per-batch groups reset each other's has_written bits and
            # drop contributions).  The first write of every other batch
            # region lands on cleared has_written bits and overwrites,
            # so start=False is correct there too.
            nc.tensor.matmul(acc_ps[:, b, :], c16s[b][:, j, :],
                             p_sb[:, b, :],
                             start=(idx == 0 and b == 0), stop=False,
                             skip_group_check=True)

    for i in range(NT + 2):
        if i < NT:
            if i % CHUNK == 0:
                load_chunk(i // CHUNK)
            if variant == "dma":
                continue
            stage_scores(i)
        if variant == "dma":
            continue
        if 1 <= i <= NT:
            stage_softmax(i - 1)
        if i >= 2 and variant in ("full",):
            stage_weighted_sum(i - 2)
    if variant == "dma":
        return

    # residual: acc_b += H.T @ uT_b  == (u_b @ H)^T
    for b in range(BPC):
        nc.tensor.matmul(acc_ps[:, b, :], h_sb, uT_sbs[b],
                         start=False, stop=(b == BPC - 1),
                         skip_group_check=True)

    acc_sb = work.tile([P, BPC, Q], f32, tag="acc_sb")
    nc.scalar.copy(acc_sb, acc_ps)
    oT_ps = ps_t.tile([Q, BPC, D], f32, tag="misc_ps")
    for b in range(BPC):
        nc.tensor.transpose(oT_ps[:, b, :], acc_sb[:, b, :], ident_f)
    o_sb = work.tile([Q, BPC, D], f32, tag="o_sb")
    nc.vector.tensor_copy(o_sb, oT_ps)
    for b in range(BPC):
        nc.sync.dma_start(o_ap[b], o_sb[:, b, :])


def _build(repeat=1, variant="full", cdt_name="f16"):
    import concourse.tile as tile
    from concourse import bacc, mybir
    from concourse.masks import make_identity

    f32 = mybir.dt.float32
    cdt = mybir.dt.float16 if cdt_name == "f16" else mybir.dt.bfloat16
    nc = bacc.Bacc("TRN2", target_bir_lowering=False, debug=False,
                   num_devices=N_CORES)

    a_ap = nc.dram_tensor("story_a", [BPC, M, D], f32, kind="ExternalInput").ap()
    u_ap = nc.dram_tensor("u", [BPC, Q, D], f32, kind="ExternalInput").ap()
    c_ap = nc.dram_tensor("story_c", [BPC, M, D], f32, kind="ExternalInput").ap()
    h_ap = nc.dram_tensor("H", [D, D], f32, kind="ExternalInput").ap()
    o_ap = nc.dram_tensor("out", [BPC, Q, D], f32, kind="ExternalOutput").ap()

    with tile.TileContext(nc) as tc:
        with (
            tc.tile_pool(name="consts", bufs=1) as consts,
            tc.tile_pool(name="io", bufs=2) as io,
            tc.tile_pool(name="work", bufs=4) as work,
            tc.tile_pool(name="small", bufs=4) as small,
            tc.tile_pool(name="dram", bufs=1, space="DRAM") as dram,
            tc.tile_pool(name="ps_t", bufs=1, space="PSUM") as ps_t,
            tc.tile_pool(name="ps_s", bufs=4, space="PSUM") as ps_s,
            tc.tile_pool(name="ps_acc", bufs=2, space="PSUM") as ps_acc,
        ):
            ident_f = consts.tile([P, P], f32)
            make_identity(nc, ident_f)
            h_f = consts.tile([D, D], f32)
            nc.sync.dma_start(h_f, h_ap)
            h_sb = consts.tile([D, D], cdt)
            nc.vector.tensor_copy(h_sb, h_f)

            pools = (consts, io, work, small, dram, ps_t, ps_s, ps_acc)
            aps = (a_ap, u_ap, c_ap, h_ap, o_ap, ident_f, h_sb)
            for _ in range(repeat):
                _emit_body(nc, tc, pools, aps, variant=variant,
                           cdt_name=cdt_name)

    nc.compile()
    return nc


def _get_nc(repeat=1, variant="full", cdt_name="f16"):
    key = ("nc", repeat, variant, cdt_name)
    if key not in _cache:
        _cache[key] = _build(repeat, variant, cdt_name)
    return _cache[key]


def _in_maps(story_a, u, story_c, H):
    story_a = np.ascontiguousarray(story_a, dtype=np.float32)
    u = np.ascontiguousarray(u, dtype=np.float32)
    story_c = np.ascontiguousarray(story_c, dtype=np.float32)
    H = np.ascontiguousarray(H, dtype=np.float32)
    maps = []
    for i in range(N_CORES):
        s = slice(i * BPC, (i + 1) * BPC)
        maps.append({
            "story_a": story_a[s],
            "u": u[s],
            "story_c": story_c[s],
            "H": H,
        })
    return maps


def kernel(story_a, u, story_c, H):
    from concourse.bass_utils import run_bass_kernel_spmd

    nc = _get_nc()
    maps = _in_maps(story_a, u, story_c, H)
    res = run_bass_kernel_spmd(nc, maps, core_ids=list(range(N_CORES)))
    out = np.concatenate([res.results[i]["out"] for i in range(N_CORES)],
                         axis=0)
    return out.astype(np.float32)
